# revision 26
# baseline (speedup 1.0000x reference)
"""Trainium2 Bass kernel for the BDH block (LN + neuron gating + causal RoPE
attention + permuted-reshape z @ encoder + residual + LN).

Sharding (8 NeuronCores): 2-way data parallel on batch x 4-way tensor
parallel. Within each 4-core group, attention is sharded by head pairs
(2 heads/core, all-reduce on attn_out), and the neuron/encoder stages are
sharded on a 1024-wide slice of each head's 4096 neurons (all-reduce after
z @ encoder).

Perf structure (v2):
- all heavy matmuls in bf16 (full PE rate; f32r streams ifmap at half rate)
- encoder slice resident in SBUF (bf16), stage D loops h-outer with a single
  PSUM accumulation over all 64 z@enc matmuls per head row-block
- dx-neuron blocks for the first heads are precomputed while allreduce1 is
  in flight, so the tensor engine never idles on the collective
- both all-reduces run on bf16 payloads; allreduce2 is chunked per head pair
  and overlapped with the remaining stage D compute
"""

import numpy as np
from ml_dtypes import bfloat16

B, T, D, H = 2, 1024, 512, 8
N = 32768
HD = D // H          # 64
NH = N // H          # 4096
EPS = 1e-5

N_CORES = 8
TPG = 4              # cores per data-parallel group
HPC = 2              # heads per core (attention sharding)
NSL = NH // TPG      # 1024: per-core slice of each head's neurons
KT = D // 128        # 4 k-tiles over D
TT = T // 128        # 8 t-tiles
V = H                # 8: t-residue factor in the permuting reshape
UP = T // V          # 128: u per head-row-block
NLB = NSL // 128     # 8 sub-blocks per head slice
NBLK = H * NLB       # 64 neuron blocks per core
NPRE = 28            # dx blocks precomputed during allreduce1

_RUNNER = None


def _host_shards(inputs):
    """Build the 8 per-core input maps from the full inputs."""
    x = np.asarray(inputs["x"], dtype=np.float32)
    Wq = np.asarray(inputs["Wq"], dtype=np.float32)
    Wk = np.asarray(inputs["Wk"], dtype=np.float32)
    Wv = np.asarray(inputs["Wv"], dtype=np.float32)
    Wo = np.asarray(inputs["Wo"], dtype=np.float32)
    dx = np.asarray(inputs["decoder_x"], dtype=np.float32)
    dy = np.asarray(inputs["decoder_y"], dtype=np.float32)
    enc = np.asarray(inputs["encoder"], dtype=np.float32)

    # rope tables in deinterleaved-row layout [128, T] (2 heads stacked; both
    # head slots share the same frequency table)
    inv_freq = 1.0 / (10000.0 ** (np.arange(0, HD, 2, dtype=np.float32) / HD))
    freqs = np.arange(T, dtype=np.float32)[:, None] * inv_freq[None, :]  # [T, 32]
    cos_t = np.cos(freqs).T  # [32, T]
    sin_t = np.sin(freqs).T
    c64 = np.concatenate([cos_t, cos_t], axis=0)            # [64, T]
    s64 = np.concatenate([-sin_t, sin_t], axis=0)           # [64, T]
    cos2 = np.concatenate([c64, c64], axis=0).astype(bfloat16)  # [128, T]
    sin2 = np.concatenate([s64, s64], axis=0).astype(bfloat16)

    # column permutations for q/k weight slices (deinterleave + swap)
    deint = np.concatenate([np.arange(0, HD, 2), np.arange(1, HD, 2)])  # [64]
    swap = np.concatenate([np.arange(32, 64), np.arange(0, 32)])        # [64]
    perm = np.concatenate([deint, HD + deint])                           # [128]
    perm_s = np.concatenate([deint[swap], HD + deint[swap]])

    # causal masks for the transposed-scores diagonal band, [4, 128, 512]
    masks = np.zeros((4, 128, 512), dtype=np.float32)
    ii = np.arange(128)[:, None]
    jj = np.arange(512)[None, :]
    for c in range(4):
        q = jj // 128
        jloc = jj % 128
        masks[c] = np.where(q < c, 0.0, np.where(q == c, (ii <= jloc).astype(np.float32), 1.0))

    ident = np.eye(128, dtype=np.float32)
    ones64 = np.ones((1, 64), dtype=np.float32)

    enc_r = enc.reshape(V, NH, D)

    in_maps = []
    for c in range(N_CORES):
        b = c // TPG
        r = c % TPG
        rows = slice(r * 128, (r + 1) * 128)   # q/k/v weight rows (2 heads)
        wqT = Wq[rows, :].T.copy()             # [512, 128]
        wkT = Wk[rows, :].T.copy()
        wvT = Wv[rows, :].T.copy()
        woT = Wo[:, rows].T.copy()             # [128, 512]
        dx_c = dx[:, :, r * NSL:(r + 1) * NSL].transpose(1, 0, 2).reshape(D, H * NSL)
        dy_c = dy[:, :, r * NSL:(r + 1) * NSL].transpose(1, 0, 2).reshape(D, H * NSL)
        # re-layout so each 128-column tile is one contiguous [128, 512] DMA:
        # dx2[nt, p, k*128+c] = dx_c[k*128+p, nt*128+c]
        dx_c = dx_c.reshape(KT, 128, NBLK, 128).transpose(2, 1, 0, 3).reshape(NBLK, 128, D)
        dy_c = dy_c.reshape(KT, 128, NBLK, 128).transpose(2, 1, 0, 3).reshape(NBLK, 128, D)
        enc_c = enc_r[:, r * NSL:(r + 1) * NSL, :].reshape(V * NSL, D)
        in_maps.append({
            "x_in": np.ascontiguousarray(x[b]),
            "wqT": np.ascontiguousarray(wqT[:, perm]).astype(bfloat16),
            "wqTs": np.ascontiguousarray(wqT[:, perm_s]).astype(bfloat16),
            "wkT": np.ascontiguousarray(wkT[:, perm]).astype(bfloat16),
            "wkTs": np.ascontiguousarray(wkT[:, perm_s]).astype(bfloat16),
            "wvT": np.ascontiguousarray(wvT).astype(bfloat16),
            "woT": np.ascontiguousarray(woT).astype(bfloat16),
            "dx_in": np.ascontiguousarray(dx_c).astype(bfloat16),
            "dy_in": np.ascontiguousarray(dy_c).astype(bfloat16),
            "enc_in": np.ascontiguousarray(enc_c).astype(bfloat16),
            "cos2": cos2,
            "sin2": sin2,
            "masks_in": masks.astype(bfloat16),
            "ident": ident.astype(bfloat16),
            "ones64": ones64.astype(bfloat16),
        })
    return in_maps


def _build_program(collectives=True, n_devices=None):
    import concourse.bacc as bacc
    import concourse.tile as tile
    from concourse import mybir

    f32 = mybir.dt.float32
    bf16 = mybir.dt.bfloat16
    AF = mybir.ActivationFunctionType

    if n_devices is None:
        n_devices = N_CORES if collectives else 1
    nc = bacc.Bacc("TRN2", target_bir_lowering=False, debug=False,
                   num_devices=n_devices)

    def all_reduce(ins_ap, outs_ap):
        if collectives:
            nc.gpsimd.collective_compute(
                "AllReduce", mybir.AluOpType.add,
                replica_groups=[[0, 1, 2, 3], [4, 5, 6, 7]],
                ins=[ins_ap], outs=[outs_ap])
        else:
            nc.sync.dma_start(out=outs_ap, in_=ins_ap)

    def din(name, shape, dt=bf16):
        return nc.dram_tensor(name, shape, dt, kind="ExternalInput").ap()

    x_in = din("x_in", [T, D], f32)
    wqT = din("wqT", [D, 128]); wqTs = din("wqTs", [D, 128])
    wkT = din("wkT", [D, 128]); wkTs = din("wkTs", [D, 128])
    wvT = din("wvT", [D, 128])
    woT = din("woT", [128, D])
    dx_in = din("dx_in", [NBLK, 128, D])
    dy_in = din("dy_in", [NBLK, 128, D])
    enc_in = din("enc_in", [V * NSL, D])
    cos2 = din("cos2", [128, T])
    sin2 = din("sin2", [128, T])
    masks_in = din("masks_in", [4, 128, 512])
    ident = din("ident", [128, 128])
    ones64 = din("ones64", [1, 64])

    y_out = nc.dram_tensor("y_out", [T, D], f32, kind="ExternalOutput").ap()

    with tile.TileContext(nc) as tc:
        with tc.tile_pool(name="const", bufs=1) as const, \
             tc.tile_pool(name="persist", bufs=1) as persist, \
             tc.tile_pool(name="dram", bufs=1, space="DRAM") as dram, \
             tc.tile_pool(name="stats", bufs=4) as stats:

            eps_t = const.tile([128, 1], f32)
            nc.vector.memset(eps_t[:], EPS)

            def ln_tile(out_ap, in_ap):
                st = stats.tile([128, 6], f32, tag="ln_st")
                nc.vector.bn_stats(out=st[:], in_=in_ap)
                mv = stats.tile([128, 2], f32, tag="ln_mv")
                nc.vector.bn_aggr(out=mv[:], in_=st[:])
                sd = stats.tile([128, 1], f32, tag="ln_sd")
                nc.scalar.activation(out=sd[:], in_=mv[:, 1:2], func=AF.Sqrt,
                                     bias=eps_t[:])
                rs = stats.tile([128, 1], f32, tag="ln_rs")
                nc.vector.reciprocal(out=rs[:], in_=sd[:])
                nc.vector.tensor_scalar(out=out_ap, in0=in_ap,
                                        scalar1=mv[:, 0:1], scalar2=rs[:],
                                        op0=mybir.AluOpType.subtract,
                                        op1=mybir.AluOpType.mult)

            # persistent SBUF tensors
            x_sb = persist.tile([128, TT, D], f32)        # x, natural [t,d]
            xnT = persist.tile([128, KT, T], bf16)        # LN(x) transposed
            lnT = persist.tile([128, KT, T], bf16)        # LN(attn) transposed
            enc_sb = persist.tile([128, V, NLB, D], bf16)  # full encoder slice
            xr_pre = persist.tile([128, NPRE, T], bf16)   # precomputed relu(x@dx)
            ident_sb = const.tile([128, 128], bf16)

            # big up-front DMAs: encoder slice + identity. The encoder load
            # rides the gpsimd queue so it doesn't delay stage A/B loads on
            # the sync queue.
            nc.sync.dma_start(out=ident_sb[:], in_=ident[:])
            for v in range(V):
                nc.gpsimd.dma_start(
                    out=enc_sb[:, v, :, :],
                    in_=enc_in[v * NSL:(v + 1) * NSL, :]
                        .rearrange("(nb p) d -> p nb d", p=128))

            # ---------------- Stage A: load x, LN, transpose ----------------
            with nc.named_scope("stageA"), \
                 tc.tile_pool(name="stA", bufs=1) as stA, \
                 tc.tile_pool(name="psA", bufs=2, space="PSUM") as psA:
                xn_sb = stA.tile([128, TT, D], bf16, tag="xn")
                for i in range(TT):
                    nc.sync.dma_start(out=x_sb[:, i, :], in_=x_in[i * 128:(i + 1) * 128, :])
                    ln_tile(xn_sb[:, i, :], x_sb[:, i, :])
                for i in range(TT):
                    for k in range(KT):
                        ps_tr = psA.tile([128, 128], bf16, tag="tr")
                        nc.tensor.transpose(ps_tr[:], xn_sb[:, i, k * 128:(k + 1) * 128], ident_sb[:])
                        nc.vector.tensor_copy(out=xnT[:, k, i * 128:(i + 1) * 128], in_=ps_tr[:])

            # ---------------- Stage B: attention (2 local heads) -------------
            # B1: q/k (roped, transposed) and v (natural)
            qrot = persist.tile([128, T], bf16)
            krot = persist.tile([128, T], bf16)
            # v with a ones column appended per head: the attention-weight
            # denominator falls out of the same matmul as attn @ v (row 64)
            v_sb = persist.tile([128, TT, 2, 65], bf16)
            nc.vector.memset(v_sb[:, :, :, 64:65], 1.0)
            with nc.named_scope("stageB1"), \
                 tc.tile_pool(name="stB1", bufs=2) as stB1, \
                 tc.tile_pool(name="wB1", bufs=1) as wB1, \
                 tc.tile_pool(name="psB1", bufs=2, space="PSUM") as psB1, \
                 tc.tile_pool(name="psV", bufs=2, space="PSUM") as psV:
                w_q = wB1.tile([128, KT, 128], bf16, tag="wq")
                w_qs = wB1.tile([128, KT, 128], bf16, tag="wqs")
                w_k = wB1.tile([128, KT, 128], bf16, tag="wk")
                w_ks = wB1.tile([128, KT, 128], bf16, tag="wks")
                w_v = wB1.tile([128, KT, 128], bf16, tag="wv")
                for (w_t, w_d) in ((w_q, wqT), (w_qs, wqTs), (w_k, wkT), (w_ks, wkTs), (w_v, wvT)):
                    nc.sync.dma_start(out=w_t[:], in_=w_d.rearrange("(k p) j -> p k j", p=128))
                cos_sb = const.tile([128, T], bf16)
                sin_sb = const.tile([128, T], bf16)
                nc.sync.dma_start(out=cos_sb[:], in_=cos2[:])
                nc.sync.dma_start(out=sin_sb[:], in_=sin2[:])

                for f in range(2):
                    tsl = slice(f * 512, (f + 1) * 512)
                    for (wa, wb, rot) in ((w_q, w_qs, qrot), (w_k, w_ks, krot)):
                        ps_a = psB1.tile([128, 512], f32, tag="ps_a")
                        ps_b = psB1.tile([128, 512], f32, tag="ps_b")
                        for k in range(KT):
                            nc.tensor.matmul(ps_a[:], wa[:, k, :], xnT[:, k, tsl],
                                             start=(k == 0), stop=(k == KT - 1))
                        for k in range(KT):
                            nc.tensor.matmul(ps_b[:], wb[:, k, :], xnT[:, k, tsl],
                                             start=(k == 0), stop=(k == KT - 1))
                        t1 = stB1.tile([128, 512], f32, tag="ropetmp1")
                        t2 = stB1.tile([128, 512], f32, tag="ropetmp2")
                        nc.vector.tensor_mul(out=t1[:], in0=ps_a[:], in1=cos_sb[:, tsl])
                        nc.vector.tensor_mul(out=t2[:], in0=ps_b[:], in1=sin_sb[:, tsl])
                        nc.vector.tensor_add(out=rot[:, tsl], in0=t1[:], in1=t2[:])
                # v natural [t, j]
                for i in range(TT):
                    ps_v = psV.tile([128, 128], f32, tag="ps_v")
                    for k in range(KT):
                        nc.tensor.matmul(ps_v[:], xnT[:, k, i * 128:(i + 1) * 128], w_v[:, k, :],
                                         start=(k == 0), stop=(k == KT - 1))
                    for h in range(2):
                        nc.vector.tensor_copy(out=v_sb[:, i, h, 0:64],
                                              in_=ps_v[:, h * 64:(h + 1) * 64])

            # B2: scores, exp, denominators, attn @ v, Wo projection.
            # attn-out rows are produced and all-reduced per T-half (chunked).
            ap_dram = [dram.tile([512, D], bf16, tag=f"ar1_in{f}", name=f"ap_dram{f}")
                       for f in range(2)]
            ar1_out = [dram.tile([512, D], bf16, tag=f"ar1_out{f}", name=f"ar1_out{f}")
                       for f in range(2)]
            with nc.named_scope("stageB2"), \
                 tc.tile_pool(name="stB2", bufs=4) as stB2, \
                 tc.tile_pool(name="stB2b", bufs=2) as stB2b, \
                 tc.tile_pool(name="wB2", bufs=1) as wB2, \
                 tc.tile_pool(name="psS", bufs=2, space="PSUM") as psS, \
                 tc.tile_pool(name="psDen", bufs=1, space="PSUM") as psDen, \
                 tc.tile_pool(name="psAv", bufs=1, space="PSUM") as psAv, \
                 tc.tile_pool(name="psAp", bufs=2, space="PSUM") as psAp:
                masks_sb = wB2.tile([128, 4, 512], bf16, tag="masks")
                nc.sync.dma_start(out=masks_sb[:], in_=masks_in.rearrange("c p n -> p c n"))
                ones_sb = wB2.tile([1, 64], bf16, tag="ones")
                nc.sync.dma_start(out=ones_sb[:], in_=ones64[:])
                wo_h = [wB2.tile([64, D], bf16, tag=f"wo{h}", name=f"wo_h{h}")
                        for h in range(2)]
                for h in range(2):
                    nc.sync.dma_start(out=wo_h[h][:], in_=woT[h * 64:(h + 1) * 64, :])

                avn = [persist.tile([64, T], bf16, name=f"avn{h}") for h in range(2)]
                av_raw = [persist.tile([64, T], f32, name=f"av_raw{h}") for h in range(2)]

                for f in range(2):
                    tsl = slice(f * 512, (f + 1) * 512)
                    # row 64 of each av accumulator is the softmax denominator
                    # (ones column of v_sb)
                    av_ps = [psAv.tile([65, 512], f32, tag=f"av{h}", name=f"av_ps{h}")
                             for h in range(2)]
                    np_tiles = 4 * f + 4
                    for p in range(np_tiles):
                        for h in range(2):
                            hsl = slice(h * 64, (h + 1) * 64)
                            s_ps = psS.tile([128, 512], f32, tag="s")
                            nc.tensor.matmul(s_ps[:], krot[hsl, p * 128:(p + 1) * 128],
                                             qrot[hsl, tsl], start=True, stop=True)
                            e_sb = stB2.tile([128, 512], bf16, tag="exp")
                            nc.scalar.activation(out=e_sb[:], in_=s_ps[:], func=AF.Exp)
                            cstar = p - 4 * f
                            if cstar >= 0:
                                nc.vector.tensor_mul(out=e_sb[:], in0=e_sb[:],
                                                     in1=masks_sb[:, cstar, :])
                            nc.tensor.matmul(av_ps[h][:], v_sb[:, p, h, :], e_sb[:],
                                             start=(p == 0), stop=(p == np_tiles - 1))
                    for h in range(2):
                        # 1/den broadcast to 64 partitions via a rank-1 matmul
                        dr = stB2b.tile([1, 512], bf16, tag="denrow")
                        with nc.allow_low_precision(reason="1/den at bf16 is ample for softmax weights"):
                            nc.vector.reciprocal(out=dr[:], in_=av_ps[h][64:65, :])
                        bc_ps = psDen.tile([64, 512], f32, tag="denbc")
                        nc.tensor.matmul(bc_ps[:], ones_sb[:], dr[:],
                                         start=True, stop=True)
                        nc.vector.tensor_copy(out=av_raw[h][:, tsl], in_=av_ps[h][0:64, :])
                        nc.vector.tensor_mul(out=avn[h][:, tsl],
                                             in0=av_raw[h][:, tsl], in1=bc_ps[:])
                    for i in range(4 * f, 4 * f + 4):
                        ap_ps = psAp.tile([128, 512], f32, tag="ap")
                        for h in range(2):
                            nc.tensor.matmul(ap_ps[:], avn[h][:, i * 128:(i + 1) * 128],
                                             wo_h[h][:], start=(h == 0), stop=(h == 1))
                        o_sb = stB2b.tile([128, 512], bf16, tag="apout")
                        nc.vector.tensor_copy(out=o_sb[:], in_=ap_ps[:])
                        nc.sync.dma_start(out=ap_dram[f][(i - 4 * f) * 128:(i - 4 * f + 1) * 128, :],
                                          in_=o_sb[:])
                    with nc.named_scope("allreduce1"):
                        all_reduce(ap_dram[f].opt(), ar1_out[f].opt())

            # ---------------- D_x: precompute relu(x @ dx) during AR1 --------
            with nc.named_scope("stageDx"), \
                 tc.tile_pool(name="wDx", bufs=3) as wDx, \
                 tc.tile_pool(name="psDx", bufs=2, space="PSUM") as psDx:
                for j in range(NPRE):
                    dxw = wDx.tile([128, KT, 128], bf16, tag="dxw")
                    nc.sync.dma_start(out=dxw[:], in_=dx_in[j, :, :].rearrange("p (k c) -> p k c", k=KT))
                    # k-outer so both T-half matmuls reuse the loaded weights
                    mm_ps = [psDx.tile([128, 512], f32, tag=f"mmx{tb}", name=f"mmx{tb}")
                             for tb in range(2)]
                    for k in range(KT):
                        for tb in range(2):
                            nc.tensor.matmul(mm_ps[tb][:], dxw[:, k, :],
                                             xnT[:, k, tb * 512:(tb + 1) * 512],
                                             start=(k == 0), stop=(k == KT - 1))
                    for tb in range(2):
                        nc.scalar.activation(out=xr_pre[:, j, tb * 512:(tb + 1) * 512],
                                             in_=mm_ps[tb][:], func=AF.Relu)

            # ---------------- Stage C: LN(attn_out), transpose ----------------
            with nc.named_scope("stageC"), \
                 tc.tile_pool(name="stC", bufs=2) as stC, \
                 tc.tile_pool(name="psC", bufs=2, space="PSUM") as psC:
                for f in range(2):
                    for i in range(4):
                        af_sb = stC.tile([128, D], bf16, tag="af")
                        nc.sync.dma_start(out=af_sb[:], in_=ar1_out[f][i * 128:(i + 1) * 128, :])
                        ln_tile(af_sb[:], af_sb[:])
                        it = 4 * f + i
                        for k in range(KT):
                            ps_tr = psC.tile([128, 128], bf16, tag="trc")
                            nc.tensor.transpose(ps_tr[:], af_sb[:, k * 128:(k + 1) * 128], ident_sb[:])
                            nc.vector.tensor_copy(out=lnT[:, k, it * 128:(it + 1) * 128], in_=ps_tr[:])

            # ---------------- Stage D: neurons, gate, z @ enc ----------------
            # h-outer loop: one PSUM bank accumulates all 64 matmuls for head
            # row-block h; allreduce2 is chunked per head pair.
            ar2_in = [dram.tile([256, D], bf16, tag=f"ar2_in{c}", name=f"ar2_in{c}")
                      for c in range(4)]
            ar2_out = [dram.tile([256, D], bf16, tag=f"ar2_out{c}", name=f"ar2_out{c}")
                       for c in range(4)]
            with nc.named_scope("stageD"), \
                 tc.tile_pool(name="wD", bufs=3) as wD, \
                 tc.tile_pool(name="actD", bufs=3) as actD, \
                 tc.tile_pool(name="outD", bufs=2) as outD, \
                 tc.tile_pool(name="psMM", bufs=3, space="PSUM") as psMM, \
                 tc.tile_pool(name="psZ", bufs=2, space="PSUM") as psZ:

                def neuron_mm(w_t, src, dst):
                    # k-outer so both T-half matmuls reuse the loaded weights
                    mm_ps = [psMM.tile([128, 512], f32, tag=f"mm{tb}", name=f"mm{tb}")
                             for tb in range(2)]
                    for k in range(KT):
                        for tb in range(2):
                            nc.tensor.matmul(mm_ps[tb][:], w_t[:, k, :],
                                             src[:, k, tb * 512:(tb + 1) * 512],
                                             start=(k == 0), stop=(k == KT - 1))
                    for tb in range(2):
                        nc.scalar.activation(out=dst[:, tb * 512:(tb + 1) * 512],
                                             in_=mm_ps[tb][:], func=AF.Relu)

                for h in range(H):
                    z_ps = psZ.tile([128, 512], f32, tag="zacc")
                    for nlb in range(NLB):
                        j = h * NLB + nlb
                        dy_t = wD.tile([128, KT, 128], bf16, tag="dy")
                        nc.sync.dma_start(out=dy_t[:], in_=dy_in[j, :, :].rearrange("p (k c) -> p k c", k=KT))
                        if j >= NPRE:
                            dx_t = wD.tile([128, KT, 128], bf16, tag="dx")
                            nc.sync.dma_start(out=dx_t[:], in_=dx_in[j, :, :].rearrange("p (k c) -> p k c", k=KT))
                            xr = actD.tile([128, T], bf16, tag="xr")
                            neuron_mm(dx_t, xnT, xr)
                            xr_ap = xr[:]
                        else:
                            xr_ap = xr_pre[:, j, :]
                        yr = actD.tile([128, T], bf16, tag="yr")
                        neuron_mm(dy_t, lnT, yr)
                        z_sb = actD.tile([128, T], bf16, tag="z")
                        nc.vector.tensor_mul(out=z_sb[:], in0=xr_ap, in1=yr[:])
                        zv = z_sb[:].rearrange("p (u v) -> p v u", v=V)
                        for v in range(V):
                            nc.tensor.matmul(z_ps[:], zv[:, v, :], enc_sb[:, v, nlb, :],
                                             start=(nlb == 0 and v == 0),
                                             stop=(nlb == NLB - 1 and v == V - 1))
                    o_sb = outD.tile([128, D], bf16, tag="oD")
                    nc.vector.tensor_copy(out=o_sb[:], in_=z_ps[:])
                    nc.sync.dma_start(out=ar2_in[h // 2][(h % 2) * 128:(h % 2 + 1) * 128, :],
                                      in_=o_sb[:])
                    if h % 2 == 1:
                        with nc.named_scope("allreduce2"):
                            all_reduce(ar2_in[h // 2].opt(), ar2_out[h // 2].opt())

            # ---------------- Final: residual + LN ----------------
            with nc.named_scope("final"), \
                 tc.tile_pool(name="stF", bufs=3) as stF:
                for c in range(4):
                    for r in range(2):
                        i = 2 * c + r
                        zt = stF.tile([128, D], bf16, tag="fz")
                        nc.sync.dma_start(out=zt[:], in_=ar2_out[c][r * 128:(r + 1) * 128, :])
                        fo = stF.tile([128, D], f32, tag="fo")
                        nc.vector.tensor_add(out=fo[:], in0=zt[:], in1=x_sb[:, i, :])
                        ln_tile(fo[:], fo[:])
                        nc.sync.dma_start(out=y_out[i * 128:(i + 1) * 128, :], in_=fo[:])

    nc.compile()
    return nc


class _Runner:
    """Compile once, jit once, execute many times."""

    def __init__(self):
        import jax
        import numpy as _np
        from jax.sharding import Mesh, PartitionSpec
        from jax.experimental.shard_map import shard_map
        from concourse import bass2jax, mybir

        self.jax = jax
        nc = _build_program()
        self.nc = nc
        bass2jax.install_neuronx_cc_hook()

        in_names, out_names, out_avals, zero_outs = [], [], [], []
        pn = nc.partition_id_tensor.name if nc.partition_id_tensor else None
        for alloc in nc.m.functions[0].allocations:
            if not isinstance(alloc, mybir.MemoryLocationSet):
                continue
            name = alloc.memorylocations[0].name
            if alloc.kind == "ExternalInput":
                if name != pn:
                    in_names.append(name)
            elif alloc.kind == "ExternalOutput":
                out_names.append(name)
                shape = tuple(alloc.tensor_shape)
                dtype = mybir.dt.np(alloc.dtype)
                out_avals.append(jax.core.ShapedArray(shape, dtype))
                zero_outs.append(_np.zeros(shape, dtype))
        self.in_names, self.out_names = in_names, out_names
        self.zero_outs = zero_outs
        n_params = len(in_names)
        all_in = in_names + out_names + ([pn] if pn else [])

        def _body(*args):
            operands = list(args)
            if pn is not None:
                operands.append(bass2jax.partition_id_tensor())
            outs = bass2jax._bass_exec_p.bind(
                *operands, out_avals=tuple(out_avals), in_names=tuple(all_in),
                out_names=tuple(out_names), lowering_input_output_aliases=(),
                sim_require_finite=True, sim_require_nnan=True, nc=nc)
            return tuple(outs)

        devices = jax.devices()[:N_CORES]
        mesh = Mesh(np.asarray(devices), ("core",))
        n_all = n_params + len(out_names)
        self.fn = jax.jit(
            shard_map(_body, mesh=mesh,
                      in_specs=(PartitionSpec("core"),) * n_all,
                      out_specs=(PartitionSpec("core"),) * len(out_names),
                      check_rep=False),
            keep_unused=True)
        self.sharding = jax.sharding.NamedSharding(mesh, PartitionSpec("core"))
        self.out_avals = out_avals

    def device_args(self, in_maps):
        concat_in = [np.concatenate([m[nm] for m in in_maps], axis=0)
                     for nm in self.in_names]
        concat_zero = [np.zeros((N_CORES * z.shape[0], *z.shape[1:]), z.dtype)
                       for z in self.zero_outs]
        return [self.jax.device_put(a, self.sharding)
                for a in concat_in + concat_zero]

    def run(self, dev_args):
        outs = self.fn(*dev_args)
        self.jax.block_until_ready(outs)
        return outs

    def results(self, outs):
        per_core = []
        for c in range(N_CORES):
            per_core.append({
                nm: np.asarray(outs[i]).reshape(N_CORES, *self.out_avals[i].shape)[c]
                for i, nm in enumerate(self.out_names)})
        return per_core


def _get_runner():
    global _RUNNER
    if _RUNNER is None:
        _RUNNER = _Runner()
    return _RUNNER


def kernel(**inputs):
    import time as _time

    in_maps = _host_shards(inputs)
    last_exc = None
    for attempt in range(3):
        try:
            runner = _get_runner()
            outs = runner.run(runner.device_args(in_maps))
            res = runner.results(outs)
            out = np.stack([res[0]["y_out"], res[TPG]["y_out"]], axis=0)
            return out.astype(np.float32)
        except Exception as exc:  # transient device/tunnel hiccups: retry once or twice
            last_exc = exc
            global _RUNNER
            _RUNNER = None
            _time.sleep(3.0)
    raise last_exc
